# revision 12
# baseline (speedup 1.0000x reference)
"""Distributed Trainium2 (8 NeuronCores) GQA attention kernel.

Problem: B=1, T=2048, D=4096, N=32 q-heads, K=8 kv-heads, H=128 (causal,
RMSNorm on q/k/v with (1+scale) on q/k, RoPE base 10000).

Sharding (tensor parallel over heads, per the hint):
  core c owns q-heads [4c, 4c+4) and kv-head c (GQA group preserved, G=4).
  x is replicated (pre-transposed + fp16 on host). Each core computes its
  heads' projections + norms + RoPE + causal attention; attention outputs
  are AllGathered in two 2-head batches per t-group, and each core computes
  the final output projection for its own 512-wide slice of D. Host
  concatenates the 8 [2048, 512] slices.

Pipeline: t is processed in 4 groups of 512, software-pipelined so the PE
never head-of-line blocks on slow chains and attention (whose outputs feed
the AllGathers) completes as early as possible each iteration:
  - per iteration (group j, head n): [deferred transposes] [logits matmuls
    zipped 1:1 with q-projection matmuls -- just enough filler to cover the
    ScalarE exp latency] [AV matmuls, unfilled -- exps are already ahead]
    [leftover q + kv projection matmuls + o-proj chunks of group j-1].
  - transposes for a projected chunk are issued one iteration later, so
    they never wait on the DVE norm/rope chain.
  - the DVE work (softmax-z accumulation x norm/rope/Newton chain) is
    interleaved thunk-by-thunk: dependent-op semaphore bubbles (~160ns
    each) are absorbed by the other chain's execution.
  - rsqrt for the RMS norms: DVE Newton iteration (11 fused ops; no ACT
    Sqrt -> no activation-table swaps; Exp/Square/Copy share one table).
  - PSUM drains and transpose copy-outs run on ScalarE (ACT), keeping DVE
    for the latency-critical chains.
  - input DMA is split across both hardware DGE queues (weights on qAct,
    x/rope/everything else on qSp).
  - the causal in-block mask is applied multiplicatively AFTER exp.

Precision: fp16 storage for x/weights/q/k/rope/out, bf16 for exp(logits)
and v (softmax is computed WITHOUT max subtraction -- max logit ~68, e^68
fits bf16 range but not fp16). Matmul accumulation f32 in PSUM; norm stats
f32; softmax denominator accumulated in bf16 on DVE + partition-reduced on
GpSimd.

Layout trick: logits are computed TRANSPOSED, lT[s,t] = kT.T @ qT, so that
exp(lT) is directly the AV-matmul rhs.
"""

import numpy as np

# ---------------------------------------------------------------- constants
T = 2048          # sequence length
D = 4096          # model dim
H = 128           # head dim
NH = 4            # q heads per core
NHEADS = 32       # total q heads
DC = 32           # d-chunks of 128 (contraction tiles)
TC = 16           # t-chunks of 128
NG = 4            # t-groups of 512 (pipeline granularity)
DSL = 512         # output D slice per core
N_CORES = 8
EPS = 1e-6
ROPE_BASE = 10000.0

_CACHE = {}


# ---------------------------------------------------------------- builder
def _build():
    import concourse.mybir as mybir
    import concourse.tile as tile
    from concourse import bacc
    from concourse import bass_isa
    from concourse.masks import make_identity

    FP16 = mybir.dt.float16
    BF16 = mybir.dt.bfloat16
    F32 = mybir.dt.float32
    Act = mybir.ActivationFunctionType
    Alu = mybir.AluOpType

    nc = bacc.Bacc("TRN2", target_bir_lowering=False, debug=False,
                   num_devices=N_CORES)

    # -------- kernel I/O (per-core shards, preprocessed on host)
    xt_d = nc.dram_tensor("xt", [TC, 128, DC, 128], FP16, kind="ExternalInput")
    wq_d = nc.dram_tensor("wq", [128, DC, NH * 128], FP16, kind="ExternalInput")
    wkv_d = nc.dram_tensor("wkv", [128, DC, 256], FP16, kind="ExternalInput")
    # wo[h, nglobal, dsl]: every head's output weights for the local D slice
    wo_d = nc.dram_tensor("wo", [128, NHEADS, DSL], FP16, kind="ExternalInput")
    cs_d = nc.dram_tensor("csp", [TC, 128, 2 * NH * 64], FP16,
                          kind="ExternalInput")
    qsb_d = nc.dram_tensor("qsb", [128, NH * 128], F32, kind="ExternalInput")
    ksb_d = nc.dram_tensor("ksb", [128, 128], F32, kind="ExternalInput")
    # 0/1 lower-triangle keep-mask (tri[s,t] = 1 iff t >= s), bf16
    tri_d = nc.dram_tensor("tri", [128, 128], BF16, kind="ExternalInput")
    out_d = nc.dram_tensor("out", [T, DSL], FP16, kind="ExternalOutput")

    rg = [list(range(N_CORES))]

    with tile.TileContext(nc) as tc:
        with (
            tc.tile_pool(name="wp", bufs=1) as wp,
            tc.tile_pool(name="xp", bufs=3) as xp,
            tc.tile_pool(name="np_", bufs=2) as np_,
            tc.tile_pool(name="pp", bufs=1) as pp,
            tc.tile_pool(name="op", bufs=2) as op,
            tc.tile_pool(name="ps", bufs=1, space="PSUM") as ps,
            tc.tile_pool(name="dr", bufs=1, space="DRAM") as dr,
        ):
            # -------- resident weights / constants
            wq_sb = wp.tile([128, DC * NH * 128], FP16, tag="wq")
            wq_flat = wq_d.ap().rearrange("p a b -> p (a b)")
            wkv_sb = wp.tile([128, DC * 256], FP16, tag="wkv")
            wkv_flat = wkv_d.ap().rearrange("p a b -> p (a b)")
            wo_sb = wp.tile([128, NHEADS * DSL], FP16, tag="wo")
            qsb_sb = wp.tile([128, NH * 128], F32, tag="qsb")
            ksb_sb = wp.tile([128, 128], F32, tag="ksb")
            tri_sb = wp.tile([128, 128], BF16, tag="tri")
            ident = wp.tile([128, 128], FP16, tag="ident")
            make_identity(nc, ident[:])

            # resident K^T [h, s], V [s, h] (fp16 / bf16), one kv head
            kT_sb = wp.tile([128, T], FP16, tag="kT")
            vf_sb = wp.tile([128, T], BF16, tag="vf")

            def load_chunk(ti):
                """Issue the input DMAs for t-chunk ti (x slab + rope)."""
                xt = xp.tile([128, DC * 128], FP16, tag="xt")
                xt_src = xt_d.ap()[ti].rearrange("p a b -> p (a b)")
                for i in range(4):
                    nc.sync.dma_start(xt[:, i * 1024:(i + 1) * 1024],
                                      xt_src[:, i * 1024:(i + 1) * 1024])
                cs_t = np_.tile([128, 2 * NH * 64], FP16, tag="cs", bufs=4)
                nc.sync.dma_start(cs_t[:], cs_d.ap()[ti])
                return xt, cs_t

            def proj_mm_lists(xt):
                """PSUM tiles + matmul thunk lists for one chunk's q/kv
                projections (zipped into the attention matmul stream)."""
                q_ps = ps.tile([128, 512], F32, tag="qps")
                kv_ps = ps.tile([128, 256], F32, tag="kvps")

                def qmm(dc):
                    nc.tensor.matmul(
                        q_ps[:], lhsT=xt[:, dc * 128:(dc + 1) * 128],
                        rhs=wq_sb[:, dc * 512:(dc + 1) * 512],
                        start=(dc == 0), stop=(dc == DC - 1))

                def kvmm(dc):
                    nc.tensor.matmul(
                        kv_ps[:], lhsT=xt[:, dc * 128:(dc + 1) * 128],
                        rhs=wkv_sb[:, dc * 256:(dc + 1) * 256],
                        start=(dc == 0), stop=(dc == DC - 1))

                q_mms = [lambda dc=dc: qmm(dc) for dc in range(DC)]
                kv_mms = [lambda dc=dc: kvmm(dc) for dc in range(DC)]
                return q_ps, kv_ps, q_mms, kv_mms

            def norm_act_q(q_ps):
                """ACT: q PSUM drain + squares. MUST be issued after all
                q-projection matmuls (tile deps are issue-ordered)."""
                q_sb = np_.tile([128, 512], F32, tag="q_sb")
                nc.scalar.copy(q_sb[:], q_ps[:])
                sq = np_.tile([128, 6], F32, tag="sq")
                scr = np_.tile([128, 128], BF16, tag="scr")
                for nq in range(NH):
                    nc.scalar.activation(
                        scr[:], q_sb[:, nq * 128:(nq + 1) * 128], Act.Square,
                        accum_out=sq[:, nq:nq + 1])
                return q_sb, sq, scr

            def norm_act_kv(kv_ps, sq, scr):
                """ACT: kv PSUM drain + squares; issue after the kv mms."""
                kv_sb = np_.tile([128, 256], F32, tag="kv_sb")
                nc.scalar.copy(kv_sb[:], kv_ps[:])
                nc.scalar.activation(scr[:], kv_sb[:, 0:128], Act.Square,
                                     accum_out=sq[:, 4:5])
                nc.scalar.activation(scr[:], kv_sb[:, 128:256], Act.Square,
                                     accum_out=sq[:, 5:6])
                return kv_sb

            def norm_chain_dve(ti, q_sb, kv_sb, sq, cs_t):
                """DVE thunks for one chunk's norm/rope chain. Returns
                (thunks, qb, kb); v slab written by the last thunk."""
                cos_t = cs_t[:, 0:NH * 64]
                sin_t = cs_t[:, NH * 64:2 * NH * 64]
                x = np_.tile([128, 6], F32, tag="nx")
                y = np_.tile([128, 6], F32, tag="ny")
                t0 = np_.tile([128, 6], F32, tag="nt")
                qa = np_.tile([128, 512], FP16, tag="qa")
                qf = np_.tile([128, 512], FP16, tag="qf")
                t1 = np_.tile([128, 256], FP16, tag="t1")
                t2 = np_.tile([128, 256], FP16, tag="t2")
                qb = np_.tile([128, 512], FP16, tag="qb")
                ka = np_.tile([128, 128], FP16, tag="ka")
                kf = np_.tile([128, 128], FP16, tag="kf")
                kb = np_.tile([128, 128], FP16, tag="kb")

                qa3 = qa[:].rearrange("p (n h) -> p n h", n=NH)
                qf3 = qf[:].rearrange("p (n h) -> p n h", n=NH)
                c3 = cos_t.rearrange("p (n h) -> p n h", n=NH)
                s3 = sin_t.rearrange("p (n h) -> p n h", n=NH)
                t13 = t1[:].rearrange("p (n h) -> p n h", n=NH)
                t23 = t2[:].rearrange("p (n h) -> p n h", n=NH)
                x1, x2 = qa3[:, :, 0:64], qa3[:, :, 64:128]

                th = []
                # q rope (independent of Newton -> issued early)
                th.append(lambda: nc.vector.tensor_mul(
                    out=qa[:], in0=q_sb[:], in1=qsb_sb[:]))
                th.append(lambda: nc.vector.tensor_mul(
                    out=t13, in0=x1, in1=c3))
                th.append(lambda: nc.vector.tensor_mul(
                    out=t23, in0=x2, in1=s3))
                th.append(lambda: nc.vector.tensor_tensor(
                    out=qf3[:, :, 0:64], in0=t13, in1=t23, op=Alu.subtract))
                th.append(lambda: nc.vector.tensor_mul(
                    out=t13, in0=x2, in1=c3))
                th.append(lambda: nc.vector.tensor_mul(
                    out=t23, in0=x1, in1=s3))
                th.append(lambda: nc.vector.tensor_tensor(
                    out=qf3[:, :, 64:128], in0=t13, in1=t23, op=Alu.add))
                # k rope
                th.append(lambda: nc.vector.tensor_mul(
                    out=ka[:], in0=kv_sb[:, 0:128], in1=ksb_sb[:]))
                th.append(lambda: nc.vector.tensor_mul(
                    out=t1[:, 0:64], in0=ka[:, 0:64], in1=cos_t[:, 0:64]))
                th.append(lambda: nc.vector.tensor_mul(
                    out=t2[:, 0:64], in0=ka[:, 64:128], in1=sin_t[:, 0:64]))
                th.append(lambda: nc.vector.tensor_tensor(
                    out=kf[:, 0:64], in0=t1[:, 0:64], in1=t2[:, 0:64],
                    op=Alu.subtract))
                th.append(lambda: nc.vector.tensor_mul(
                    out=t1[:, 0:64], in0=ka[:, 64:128], in1=cos_t[:, 0:64]))
                th.append(lambda: nc.vector.tensor_mul(
                    out=t2[:, 0:64], in0=ka[:, 0:64], in1=sin_t[:, 0:64]))
                th.append(lambda: nc.vector.tensor_tensor(
                    out=kf[:, 64:128], in0=t1[:, 0:64], in1=t2[:, 0:64],
                    op=Alu.add))
                # Newton rsqrt: x' = -0.5*(sq/H + eps); y0 = 1.5 + x';
                # y <- (x'*y^2 + 1.5) * y, 3 times.
                th.append(lambda: nc.vector.tensor_scalar(
                    out=x[:], in0=sq[:], scalar1=-0.5 / H, scalar2=-0.5 * EPS,
                    op0=Alu.mult, op1=Alu.add))
                th.append(lambda: nc.vector.tensor_scalar_add(
                    out=y[:], in0=x[:], scalar1=1.5))
                for _ in range(3):
                    th.append(lambda: nc.vector.tensor_mul(
                        out=t0[:], in0=y[:], in1=y[:]))
                    th.append(lambda: nc.vector.tensor_mul(
                        out=t0[:], in0=t0[:], in1=x[:]))
                    th.append(lambda: nc.vector.scalar_tensor_tensor(
                        out=y[:], in0=t0[:], scalar=1.5, in1=y[:],
                        op0=Alu.add, op1=Alu.mult))
                # final scales
                for nq in range(NH):
                    th.append(lambda nq=nq: nc.vector.tensor_scalar_mul(
                        out=qb[:, nq * 128:(nq + 1) * 128],
                        in0=qf[:, nq * 128:(nq + 1) * 128],
                        scalar1=y[:, nq:nq + 1]))
                th.append(lambda: nc.vector.tensor_scalar_mul(
                    out=kb[:], in0=kf[:], scalar1=y[:, 4:5]))
                th.append(lambda: nc.vector.tensor_scalar_mul(
                    out=vf_sb[:, ti * 128:(ti + 1) * 128],
                    in0=kv_sb[:, 128:256], scalar1=y[:, 5:6]))
                return th, qb, kb

            def do_transposes(qb, kb, qT_dst, ti, tl):
                """PE transposes of one chunk's qb/kb into the resident
                qT / kT slabs (issued one iteration after the chain);
                copy-outs on ACT."""
                for nq in range(NH):
                    tp = ps.tile([128, 128], FP16, tag="tp")
                    nc.tensor.transpose(
                        tp[:], qb[:, nq * 128:(nq + 1) * 128], ident[:])
                    nc.scalar.copy(
                        qT_dst[:, nq * 512 + tl * 128:
                               nq * 512 + (tl + 1) * 128], tp[:])
                tp = ps.tile([128, 128], FP16, tag="tp")
                nc.tensor.transpose(tp[:], kb[:], ident[:])
                nc.scalar.copy(kT_sb[:, ti * 128:(ti + 1) * 128], tp[:])

            def attn_lt_phase(j, n, qT, fillers):
                """Logits matmuls zipped 1:1 with filler matmuls (just
                enough to cover exp latency) + exp + post-exp mask."""
                nk = 4 * (j + 1)
                pT_tiles = []
                fi = 0
                for k in range(nk):
                    dcol = k - 4 * j
                    lo = max(dcol, 0) * 128
                    lt = ps.tile([128, 512], F32, tag="lt", bufs=3)
                    nc.tensor.matmul(
                        lt[:, lo:512], lhsT=kT_sb[:, k * 128:(k + 1) * 128],
                        rhs=qT[:, n * 512 + lo:(n + 1) * 512],
                        start=True, stop=True)
                    if fi < len(fillers):
                        fillers[fi]()
                        fi += 1
                    pT_k = pp.tile([128, 512], BF16, tag=f"pT{k}")
                    nc.scalar.activation(pT_k[:, lo:512], lt[:, lo:512],
                                         Act.Exp)
                    if dcol >= 0:
                        nc.vector.tensor_mul(out=pT_k[:, lo:lo + 128],
                                             in0=pT_k[:, lo:lo + 128],
                                             in1=tri_sb[:])
                    pT_tiles.append((pT_k, lo))
                return pT_tiles, fi

            def attn_av_mms(pT_tiles):
                """AV matmuls (unfilled; exps are ahead by the lt phase)."""
                nk = len(pT_tiles)
                av = ps.tile([128, 512], F32, tag="av")
                for k in range(nk):
                    pk, lo = pT_tiles[k]
                    nc.tensor.matmul(av[:, lo:512],
                                     lhsT=vf_sb[:, k * 128:(k + 1) * 128],
                                     rhs=pk[:, lo:512],
                                     start=(k == 0), stop=(k == nk - 1))
                return av

            def z_chain_dve(pT_tiles, av):
                """DVE thunks for the softmax-z chain; GpSimd ops are
                issued by the thunks at the right points. Returns
                (thunks, outT)."""
                nk = len(pT_tiles)
                zacc = np_.tile([128, 512], BF16, tag="zacc", bufs=1)
                zsum = np_.tile([128, 512], F32, tag="zsum", bufs=1)
                rz = np_.tile([1, 512], F32, tag="rz", bufs=1)
                bz = np_.tile([128, 512], F32, tag="bz", bufs=1)
                outT = op.tile([128, 512], FP16, tag="outT", bufs=3)
                th = []
                p0, lo0 = pT_tiles[0]
                p1, lo1 = pT_tiles[1]
                th.append(lambda: nc.vector.tensor_tensor(
                    out=zacc[:, lo1:512], in0=p0[:, lo1:512],
                    in1=p1[:, lo1:512], op=Alu.add))
                if lo1 > 0:
                    th.append(lambda: nc.vector.tensor_copy(
                        out=zacc[:, 0:lo1], in_=p0[:, 0:lo1]))
                for k in range(2, nk):
                    pk, lo = pT_tiles[k]
                    th.append(lambda pk=pk, lo=lo: nc.vector.tensor_tensor(
                        out=zacc[:, lo:512], in0=zacc[:, lo:512],
                        in1=pk[:, lo:512], op=Alu.add))

                def reduce_and_recip():
                    nc.gpsimd.partition_all_reduce(
                        zsum[:], zacc[:], 128, bass_isa.ReduceOp.add)
                    nc.vector.reciprocal(rz[:], zsum[0:1, :])
                    nc.gpsimd.partition_broadcast(bz[:], rz[:])
                th.append(reduce_and_recip)
                th.append(lambda: nc.vector.tensor_mul(
                    out=outT[:], in0=av[:], in1=bz[:]))
                return th, outT

            def dve_zip(a, b):
                """Interleave two DVE thunk lists (absorbs the ~160ns
                dependent-op semaphore bubbles of each serial chain)."""
                ia = ib = 0
                while ia < len(a) or ib < len(b):
                    if ia < len(a):
                        a[ia]()
                        ia += 1
                    if ib < len(b):
                        b[ib]()
                        ib += 1

            def gather_heads(j, p, outTs, heads):
                """AllGather the given local heads' outputs; returns the
                gathered DRAM tile [core, len(heads), 128, 512]."""
                nh = len(heads)
                ag_in = dr.tile([nh, 128, 512], FP16, tag=f"agin{j}_{p}")
                for i, n in enumerate(heads):
                    nc.sync.dma_start(ag_in[i], outTs[n][:])
                ag_out = dr.tile([N_CORES, nh, 128, 512], FP16,
                                 tag=f"agout{j}_{p}", addr_space="Shared")
                nc.gpsimd.collective_compute(
                    "AllGather", Alu.bypass, replica_groups=rg,
                    ins=[ag_in.rearrange("a b c -> (a b c)")],
                    outs=[ag_out.rearrange("a b c d -> (a b c d)")])
                return ag_out, heads

            def agt_load(ti, ag, tag, bufs):
                """Prefetch one gathered head-batch's t-chunk slices into
                SBUF (issued well before the o-proj matmuls consume them)."""
                ag_out, heads = ag
                nh = len(heads)
                agt = op.tile([128, nh * N_CORES * 128], FP16, tag=tag,
                              bufs=bufs, name=tag)
                nc.sync.dma_start(
                    agt[:].rearrange("p (a b c) -> p a b c", a=N_CORES, b=nh),
                    ag_out.rearrange("c i h t -> h c i t")
                    [:, :, :, (ti % 4) * 128:(ti % 4 + 1) * 128])
                return agt, heads

            def oproj_slot_thunks(loaded, o_ps, start, stop):
                """Thunks for one gathered head-batch's o-proj matmuls."""
                agt, heads = loaded
                nh = len(heads)
                thunks = []
                for c8 in range(N_CORES):
                    for i, n in enumerate(heads):
                        nhead = 4 * c8 + n

                        def mm(c8=c8, i=i, nhead=nhead):
                            nc.tensor.matmul(
                                o_ps[:],
                                lhsT=agt[:, (c8 * nh + i) * 128:
                                         (c8 * nh + i + 1) * 128],
                                rhs=wo_sb[:, nhead * 512:(nhead + 1) * 512],
                                start=(start and c8 == 0 and i == 0),
                                stop=(stop and c8 == N_CORES - 1
                                      and i == nh - 1))
                        thunks.append(mm)
                return thunks

            def oproj_prefetch(j, tl, ags):
                # batch A loads early; batch B is loaded at drain time (its
                # AllGather finishes later -- a waiting dma_start would
                # head-of-line block the whole Sync DMA queue).
                ti = 4 * j + tl
                return agt_load(ti, ags[0], "agtA", 2)

            def oproj_chunk_thunks(j, tl, loadA, sags):
                """Returns (mm_thunks, finalize) for one output chunk.
                Loads batch B here (drain time)."""
                ti = 4 * j + tl
                loadB = agt_load(ti, sags[1], "agtB", 2)
                o_ps = ps.tile([128, 512], F32, tag="ops")
                thunks = (oproj_slot_thunks(loadA, o_ps, True, False)
                          + oproj_slot_thunks(loadB, o_ps, False, True))

                def finalize():
                    o_sb = op.tile([128, 512], FP16, tag="osb", bufs=1)
                    nc.scalar.copy(o_sb[:], o_ps[:])
                    nc.sync.dma_start(
                        out_d.ap()[ti * 128:(ti + 1) * 128, :], o_sb[:])
                return thunks, finalize

            def oproj_tail(j, ags):
                """Last group: batch 0 (heads 0-1, gathered earlier) for all
                4 chunks first, then batch 1 (heads 2-3) as a second PSUM
                pass merged with a DVE add, so the final AllGather's latency
                is covered by batch-0 matmuls."""
                partials = []
                for tl in range(4):
                    la = agt_load(4 * j + tl, ags[0], "agtA", 2)
                    o_ps = ps.tile([128, 512], F32, tag="ops")
                    for f in oproj_slot_thunks(la, o_ps, True, True):
                        f()
                    o_sbp = op.tile([128, 512], FP16, tag="osbp", bufs=4,
                                    name="osbp")
                    nc.scalar.copy(o_sbp[:], o_ps[:])
                    partials.append(o_sbp)
                for tl in range(4):
                    ti = 4 * j + tl
                    lb = agt_load(ti, ags[1], "agtB", 2)
                    o_ps = ps.tile([128, 512], F32, tag="ops")
                    for f in oproj_slot_thunks(lb, o_ps, True, True):
                        f()
                    o_sb2 = op.tile([128, 512], FP16, tag="osb", bufs=1)
                    nc.vector.tensor_tensor(out=o_sb2[:], in0=o_ps[:],
                                            in1=partials[tl][:], op=Alu.add)
                    nc.sync.dma_start(
                        out_d.ap()[ti * 128:(ti + 1) * 128, :], o_sb2[:])

            # -------- software pipeline. Warm-up matmuls keep the PE busy
            # through the DMA lead-in so the HAM clock gate unthrottles
            # (1.2 -> 2.4 GHz) before the real work arrives.
            chunks = {}
            qT_cur = np_.tile([128, NH * 512], FP16, tag="qT", name="qT")
            wsb = wp.tile([128, 512], FP16, tag="warm", name="warm")
            nc.vector.memset(wsb[:], 0.0)
            for i in range(32):
                wps = ps.tile([128, 512], F32, tag="lt", bufs=3)
                nc.tensor.matmul(wps[:], lhsT=wsb[:, 0:128], rhs=wsb[:],
                                 start=True, stop=True)

            # startup loads: weights on the ACT hardware-DGE queue (Scalar
            # is idle early), x/rope/scales on the SP queue -- both queues
            # stream in parallel.
            for i in range(16):
                nc.scalar.dma_start(wq_sb[:, i * 1024:(i + 1) * 1024],
                                    wq_flat[:, i * 1024:(i + 1) * 1024])
            chunks[0] = load_chunk(0)
            nc.sync.dma_start(qsb_sb[:], qsb_d.ap())
            nc.sync.dma_start(ksb_sb[:], ksb_d.ap())
            nc.sync.dma_start(tri_sb[:], tri_d.ap())
            chunks[1] = load_chunk(1)
            for i in range(4):
                nc.scalar.dma_start(wkv_sb[:, i * 2048:(i + 1) * 2048],
                                    wkv_flat[:, i * 2048:(i + 1) * 2048])
            chunks[2] = load_chunk(2)

            # group 0 projections (no attention yet); transposes deferred
            # one chunk so they never wait on the norm chain.
            pend_txp = None
            for tl in range(4):
                xt, cs_t = chunks.pop(tl)
                q_ps, kv_ps, q_mms, kv_mms = proj_mm_lists(xt)
                for f in q_mms:
                    f()
                if pend_txp is not None:
                    do_transposes(*pend_txp)
                for f in kv_mms:
                    f()
                if tl + 3 < TC:
                    chunks[tl + 3] = load_chunk(tl + 3)
                q_sb, sq, scr = norm_act_q(q_ps)
                kv_sb = norm_act_kv(kv_ps, sq, scr)
                nth, qb, kb = norm_chain_dve(tl, q_sb, kv_sb, sq, cs_t)
                for f in nth:
                    f()
                pend_txp = (qb, kb, qT_cur, tl, tl)
            wo_flat = wo_d.ap().rearrange("p a b -> p (a b)")
            for i in range(8):
                nc.scalar.dma_start(wo_sb[:, i * 2048:(i + 1) * 2048],
                                    wo_flat[:, i * 2048:(i + 1) * 2048])

            # o-proj chunks are consumed with a one-iteration shift
            # (0/1/1/2 per iteration); prefetches run one chunk per
            # iteration (1/2/3/4 cumulative) so the data is in SBUF an
            # iteration before the matmuls.
            oproj_q = []      # (src_group, chunk, ags) awaiting prefetch
            pend = []         # prefetched, awaiting matmuls
            for j in range(NG):
                qT_next = (np_.tile([128, NH * 512], FP16, tag="qT",
                                    name="qT") if j + 1 < NG else None)
                outTs = []
                ags = []
                batches = ([0, 1], [2, 3])
                prefetched = 0
                for n in range(NH):
                    while oproj_q and prefetched < (1, 2, 3, 4)[n]:
                        js, tl, sags = oproj_q.pop(0)
                        pend.append((js, tl, oproj_prefetch(js, tl, sags),
                                     sags))
                        prefetched += 1
                    od_jobs = []
                    for _ in range((0, 1, 1, 2)[n]):
                        if pend:
                            od_jobs.append(pend.pop(0))
                    if j + 1 < NG:
                        ti = 4 * (j + 1) + n
                        xt, cs_t = chunks.pop(ti)
                        q_ps, kv_ps, q_mms, kv_mms = proj_mm_lists(xt)
                        if ti + 3 < TC:
                            chunks[ti + 3] = load_chunk(ti + 3)
                    else:
                        q_ps = kv_ps = cs_t = None
                        q_mms, kv_mms = [], []
                    if pend_txp is not None:
                        do_transposes(*pend_txp)
                        pend_txp = None

                    if j + 1 < NG:
                        # attention first: lt zipped 1:1 with q-proj,
                        # AV unfilled, then the rest of the projections
                        # and the o-proj chunks.
                        pTs, fi = attn_lt_phase(j, n, qT_cur, q_mms)
                        av = attn_av_mms(pTs)
                        for f in q_mms[fi:]:
                            f()
                        q_sb, sq, scr = norm_act_q(q_ps)
                        for f in kv_mms:
                            f()
                        kv_sb = norm_act_kv(kv_ps, sq, scr)
                        for job in od_jobs:
                            th, fin = oproj_chunk_thunks(*job)
                            for f in th:
                                f()
                            fin()
                        zth, outT = z_chain_dve(pTs, av)
                        outTs.append(outT)
                        nth, qb, kb = norm_chain_dve(
                            4 * (j + 1) + n, q_sb, kv_sb, sq, cs_t)
                        dve_zip(zth, nth)
                        pend_txp = (qb, kb, qT_next, 4 * (j + 1) + n, n)
                    else:
                        # last group: o-proj matmuls are the lt fillers
                        # (their agt data was prefetched an iteration ago).
                        if len(od_jobs) >= 1:
                            thA, finA = oproj_chunk_thunks(*od_jobs[0])
                            lt_fill = thA + [finA]
                        else:
                            lt_fill = []
                        pTs, fi = attn_lt_phase(j, n, qT_cur, lt_fill)
                        av = attn_av_mms(pTs)
                        for f in lt_fill[fi:]:
                            f()
                        if len(od_jobs) == 2:
                            thB, finB = oproj_chunk_thunks(*od_jobs[1])
                            for f in thB:
                                f()
                            finB()
                        zth, outT = z_chain_dve(pTs, av)
                        outTs.append(outT)
                        dve_zip(zth, [])
                    for p, b in enumerate(batches):
                        if n == b[-1]:
                            ags.append(gather_heads(j, p, outTs, list(b)))
                for tl in range(4):
                    oproj_q.append((j, tl, ags))
                if j == NG - 1:
                    last_ags = ags
                qT_cur = qT_next
            oproj_tail(NG - 1, last_ags)

    nc.compile()
    return nc


def _get_nc():
    if "nc" not in _CACHE:
        _CACHE["nc"] = _build()
    return _CACHE["nc"]


# ---------------------------------------------------------------- host prep
def _make_in_maps(x, segment_pos, attn_mask, q_w, kv_w, o_w, q_scale, k_scale):
    x = np.asarray(x, np.float32)
    q_w = np.asarray(q_w, np.float32)
    kv_w = np.asarray(kv_w, np.float32)
    o_w = np.asarray(o_w, np.float32)
    q_scale = np.asarray(q_scale, np.float32)
    k_scale = np.asarray(k_scale, np.float32)
    pos = np.asarray(segment_pos)[0].astype(np.float32)

    x2 = x[0]  # [T, D]
    # xt[ti, p, dc, tl] = x[ti*128+tl, dc*128+p]
    xt = np.ascontiguousarray(
        x2.reshape(TC, 128, DC, 128).transpose(0, 3, 2, 1)).astype(np.float16)

    frac = 2.0 * np.arange(H // 2, dtype=np.float32) / H
    ts_ = (ROPE_BASE ** frac).astype(np.float32)
    sinu = pos[:, None] / ts_[None, :]          # [T, 64]
    csp = np.concatenate([np.tile(np.cos(sinu), (1, NH)),
                          np.tile(np.sin(sinu), (1, NH))],
                         axis=1).astype(np.float16).reshape(
        TC, 128, 2 * NH * 64)

    # tri[s, t] = 1 where t >= s (in-block causal keep mask, post-exp)
    import ml_dtypes
    tri = np.triu(np.ones((128, 128), np.float32)).astype(ml_dtypes.bfloat16)

    qs_row = np.tile(1.0 + q_scale, NH)                       # [512]
    qsb = np.ascontiguousarray(
        np.broadcast_to(qs_row[None, :], (128, NH * 128))).astype(np.float32)
    ksb = np.ascontiguousarray(
        np.broadcast_to((1.0 + k_scale)[None, :], (128, 128))).astype(
            np.float32)

    in_maps = []
    for c in range(N_CORES):
        qw_c = q_w[NH * c:NH * (c + 1)]           # [4, D, H]
        # wq[p, dc, n*128+h] = qw_c[n, dc*128+p, h]
        wq = np.ascontiguousarray(
            qw_c.transpose(1, 0, 2).reshape(DC, 128, NH * H).transpose(
                1, 0, 2)).astype(np.float16)
        kv_c = kv_w[:, c]                         # [2, D, H]
        wkv = np.ascontiguousarray(
            kv_c.transpose(1, 0, 2).reshape(DC, 128, 2 * H).transpose(
                1, 0, 2)).astype(np.float16)
        # wo[h, n, dsl] = o_w[n, h, c*512 + dsl]
        wo = np.ascontiguousarray(
            o_w[:, :, DSL * c:DSL * (c + 1)].transpose(1, 0, 2)).astype(
                np.float16)
        in_maps.append({
            "xt": xt, "wq": wq, "wkv": wkv, "wo": wo,
            "csp": csp, "qsb": qsb, "ksb": ksb,
            "tri": tri,
        })
    return in_maps


def _execute(in_maps, trace=False):
    from concourse import bass_utils
    nc = _get_nc()
    return bass_utils.run_bass_kernel_spmd(
        nc, in_maps, core_ids=list(range(N_CORES)), trace=trace)


# ---------------------------------------------------------------- entry
def kernel(x, segment_pos, attn_mask, q_w, kv_w, o_w, q_scale, k_scale):
    in_maps = _make_in_maps(x, segment_pos, attn_mask, q_w, kv_w, o_w,
                            q_scale, k_scale)
    res = _execute(in_maps, trace=False)
    outs = [np.asarray(res.results[c]["out"]) for c in range(N_CORES)]
    full = np.concatenate(outs, axis=1).astype(np.float32)
    return full[None]
